# revision 38
# baseline (speedup 1.0000x reference)
"""Bass/Trainium2 kernel for nn_CausalWeighting — fp16, two gather paths, v2.

Per batch b (one core per batch):
    s = sigmoid(relu(A[src] + C[dst]) @ w2 + b2)
with tables prescaled by |w2| and hidden dims permuted so paired dims
(d, d+128) share sign(w2); the w2 dot then reduces with a per-partition
sign vector, enabling a fused relu+add (scalar_tensor_tensor) and a single
matmul per 128-slot group on gamma "Z" windows.

Paths:
  * alpha: [128, Ga] grid bucketed by src%128, packed-int32 ap_gather from
    feature-major tables, DVE add+relu, sign-dot matmul pairs -> sa_ps[q, g].
  * gamma: one-hot (fp8) matmuls accumulate A'+C' in PSUM windows; relu via
    Act/DVE; scores land on arbitrary partitions and are routed via
    transpose -> local_scatter panels -> panel transposes into scat.
  * 4 local_scatters build the dense [512,512] fp16 + upconvert + DMA out.

v2 changes vs baseline: single act-table load at t~0 (dummy sigmoid),
merged param DMA blobs, fp8 one-hots in window-chunks, TS=2048 gathers,
Ga rebalanced 72->48, sign-pair/|w2|-prescale trick, Z-windows with fused
STT relu+add and single w2 matmuls.

Sharding: data-parallel over batch, one batch per core (8 cores).
"""

import sys

import numpy as np
import ml_dtypes

if "/opt/trn_rl_repo" not in sys.path:
    sys.path.insert(0, "/opt/trn_rl_repo")

import concourse.bacc as bacc
import concourse.mybir as mybir
import concourse.tile as tile
from concourse.bass_utils import run_bass_kernel_spmd

B, N, D, E = 8, 512, 256, 16384
NCORES = 8
TS = 2048            # alpha slots per tile
WIN = 512            # gamma slots per window
GA_TARGET = 56       # alpha columns (x128 slots)
ZFRAC = 0.7          # fraction of gamma windows using the fused Z path
OH_CHUNKS = 4        # one-hot DMA chunks
FP32 = mybir.dt.float32
FP16 = mybir.dt.float16
FP8 = mybir.dt.float8e4
I32 = mybir.dt.int32
I16 = mybir.dt.int16
AF = mybir.ActivationFunctionType
ALU = mybir.AluOpType

_cache = {}


def build_program(cfg):
    """cfg: (Ga, Sg, mu, W, segs) with segs = tuple of
    (window, col_lo, col_hi, c1, c2) covering [0, Sg)."""
    Ga, Sg, mu, W, segs = cfg
    Sa = 128 * Ga
    assert Sg % WIN == 0
    nwin = Sg // WIN
    Gg = Sg // 128
    assert Gg <= 128
    na = Sa // TS
    nc = bacc.Bacc(None)

    # blob1a layout (int16 columns): critical-path params
    SW = Sa // 16
    o_id = 0          # ident fp16 [128, 128]
    o_w1 = 128        # w1 fp16 [128, 4*256]
    o_b1c = 1152      # b1c fp32 [128, 2] -> 4 cols
    o_b2 = 1156       # b2 fp32 [128, 1] -> 2 cols
    o_sgn = 1158      # sgn fp16 [128, 2] (two equal cols)
    NB1A = 1160
    # blob1b: b1r + gather indices
    o_b1r = 0         # b1r fp16 [128, 256]
    o_srcw = 256      # i16 [128, SW]
    o_dstw = o_srcw + SW
    NB1B = o_dstw + SW
    # blob2 layout
    o_rt1 = 0
    o_ls = 128
    NB2 = 128 + 4 * W

    ft_ext = nc.declare_dram_parameter("ft", [128, 2, N], FP16, isOutput=False)
    b1a_ext = nc.declare_dram_parameter("blob1a", [128, NB1A], I16, isOutput=False)
    b1b_ext = nc.declare_dram_parameter("blob1b", [128, NB1B], I16, isOutput=False)
    b2_ext = nc.declare_dram_parameter("blob2", [128, NB2], I16, isOutput=False)
    ohA_ext = nc.declare_dram_parameter("ohA", [128, Sg], FP8, isOutput=False)
    ohC_ext = nc.declare_dram_parameter("ohC", [128, Sg], FP8, isOutput=False)
    out_ext = nc.declare_dram_parameter("out", [N, N], FP32, isOutput=True)

    # window strategies: True = Z (fused STT+single matmuls); P windows
    # (both relus on Act) are assigned next to alpha DVE pieces, below.
    wz = [True] * nwin

    segs_by_win = [[] for _ in range(nwin)]
    for (w, lo, hi, c1, c2) in segs:
        segs_by_win[w].append((lo, hi, c1, c2))

    # alpha tile sizes: trailing tiles smaller for a short tail
    assert Sa % 1024 == 0
    nbig = max(0, (Sa - 2048) // 2048)
    tsz = [2048] * nbig + [1024] * ((Sa - 2048 * nbig) // 1024)
    t0s = np.cumsum([0] + tsz).tolist()
    nat = len(tsz)

    from contextlib import ExitStack
    with tile.TileContext(nc) as tc:
        with (
            tc.tile_pool(name="const", bufs=1) as cpool,
            tc.tile_pool(name="work", bufs=2) as wpool,
            tc.tile_pool(name="gwork", bufs=nat) as gpool,
            tc.tile_pool(name="hwork", bufs=4) as hpool,
        ):
            # ---- act-table preload: dummy sigmoid first ----
            dummy = cpool.tile([128, 8], FP16)
            nc.vector.memset(dummy[:], 0)
            nc.scalar.activation(dummy[:], dummy[:], AF.Sigmoid)

            # ---- PE warmup: dep-free dummy matmuls ramp the p-state so the
            # real table matmuls start at full clock (3us continuous rule)
            wmt = cpool.tile([128, N], FP16)
            nc.vector.memset(wmt[:], 0)

            # ---- loads ----
            ft_sb = cpool.tile([128, 2, N], FP16)
            nc.sync.dma_start(out=ft_sb[:], in_=ft_ext[:])
            bl1a = cpool.tile([128, NB1A], I16)
            nc.sync.dma_start(out=bl1a[:], in_=b1a_ext[:])
            bl1b = cpool.tile([128, NB1B], I16)
            nc.sync.dma_start(out=bl1b[:], in_=b1b_ext[:])
            ohA_sb = cpool.tile([128, Sg], FP8)
            ohC_sb = cpool.tile([128, Sg], FP8)
            ch = (nwin + OH_CHUNKS - 1) // OH_CHUNKS
            for c0 in range(0, nwin, ch):
                sl = slice(c0 * WIN, min((c0 + ch) * WIN, Sg))
                nc.sync.dma_start(out=ohA_sb[:, sl], in_=ohA_ext[:, sl])
                nc.sync.dma_start(out=ohC_sb[:, sl], in_=ohC_ext[:, sl])
            bl2 = cpool.tile([128, NB2], I16)
            nc.sync.dma_start(out=bl2[:], in_=b2_ext[:])

            id_sb = bl1a[:, o_id:o_id + 128].bitcast(FP16)
            b1c_sb = bl1a[:, o_b1c:o_b1c + 4].bitcast(FP32)
            b2_sb = bl1a[:, o_b2:o_b2 + 2].bitcast(FP32)
            sgn_sb = bl1a[:, o_sgn:o_sgn + 2].bitcast(FP16)
            b1r_sb = bl1b[:, o_b1r:o_b1r + 256].bitcast(FP16)
            srcw_sb = bl1b[:, o_srcw:o_srcw + SW]
            dstw_sb = bl1b[:, o_dstw:o_dstw + SW]
            rt1_sb = bl2[:, o_rt1:o_rt1 + 128]
            ls_sb = bl2[:, o_ls:o_ls + 4 * W]

            def w1_blk(k, lo, hi):
                return bl1a[:, o_w1 + k * 256 + lo:o_w1 + k * 256 + hi].bitcast(FP16)

            setup_ps = ExitStack()
            pspool = setup_ps.enter_context(
                tc.tile_pool(name="ps", bufs=2, space="PSUM"))

            for _ in range(7):
                wps = pspool.tile([128, N], FP32, tag="tb")
                nc.tensor.matmul(wps[:], lhsT=wmt[:, 0:128], rhs=wmt[:],
                                 start=True, stop=True)

            # ---- feature-major packed tables (alpha); at16 first: it gates
            # the gathers ----
            at16 = cpool.tile([128, 2 * N], FP16)
            ct16 = cpool.tile([128, 2 * N], FP16)
            for m in range(2):
                psa = pspool.tile([128, N], FP32, tag="tb")
                for k in range(2):
                    nc.tensor.matmul(psa[:], lhsT=w1_blk(k, m * 128, (m + 1) * 128),
                                     rhs=ft_sb[:, k, :], start=(k == 0), stop=(k == 1))
                nc.scalar.activation(at16[:, m::2], psa[:], AF.Identity,
                                     bias=b1c_sb[:, m:m + 1])
            for m in range(2):
                psc = pspool.tile([128, N], FP32, tag="tb")
                for k in range(2):
                    nc.tensor.matmul(psc[:], lhsT=w1_blk(2 + k, m * 128, (m + 1) * 128),
                                     rhs=ft_sb[:, k, :], start=(k == 0), stop=(k == 1))
                nc.scalar.activation(ct16[:, m::2], psc[:], AF.Identity)

            # ---- node-major tables in SBUF (gamma lhsT) ----
            san = cpool.tile([128, 4, D], FP16)
            scn = cpool.tile([128, 4, D], FP16)
            for c in range(4):
                psn = pspool.tile([128, D], FP32, tag="nm")
                for k in range(2):
                    nc.tensor.matmul(psn[:], lhsT=ft_sb[:, k, c * 128:(c + 1) * 128],
                                     rhs=w1_blk(k, 0, 256), start=(k == 0), stop=(k == 1))
                nc.vector.tensor_add(san[:, c, :], psn[:], b1r_sb)
                psn2 = pspool.tile([128, D], FP32, tag="nm")
                for k in range(2):
                    nc.tensor.matmul(psn2[:], lhsT=ft_sb[:, k, c * 128:(c + 1) * 128],
                                     rhs=w1_blk(2 + k, 0, 256), start=(k == 0), stop=(k == 1))
                nc.vector.tensor_copy(scn[:, c, :], psn2[:])
            atP = at16[:].bitcast(I32)
            ctP = ct16[:].bitcast(I32)

            setup_ps.close()
            psspool = tc.alloc_tile_pool(name="pss", bufs=1, space="PSUM")
            sa_tile = psspool.tile([128, max(Ga, 2)], FP32, tag="sa")
            sg_tile = psspool.tile([128, max(Gg, 2)], FP32, tag="sg")
            sa_ps = sa_tile[:]
            sg_ps = sg_tile[:]
            psw = tc.alloc_tile_pool(name="psw", bufs=3, space="PSUM")

            alpha_relu = []

            alpha_g = []

            def ap_tile(i):
                t0, ts = t0s[i], tsz[i]
                csl = slice(t0 // 16, (t0 + ts) // 16)
                ga = gpool.tile([128, TS], I32, tag="ga")
                gc = gpool.tile([128, TS], I32, tag="gc")
                nc.gpsimd.ap_gather(ga[:, 0:ts], atP, srcw_sb[:, csl],
                                    channels=128, num_elems=N, d=1, num_idxs=ts)
                nc.gpsimd.ap_gather(gc[:, 0:ts], ctP, dstw_sb[:, csl],
                                    channels=128, num_elems=N, d=1, num_idxs=ts)
                ga16 = ga[:, 0:ts].bitcast(FP16)
                gc16 = gc[:, 0:ts].bitcast(FP16)
                nc.vector.tensor_add(ga16, ga16, gc16)
                nc.vector.tensor_scalar_max(ga16, ga16, 0.0)
                alpha_relu.append(ga16)

            def ap_dots(i):
                # deferred: emitted after all windows so the in-order PE
                # stream never stalls on DVE behind these
                t0, ts = t0s[i], tsz[i]
                ga16 = alpha_relu[i]
                for b in range(ts // 128):
                    g = t0 // 128 + b
                    nc.tensor.matmul(sa_ps[:, g:g + 1],
                                     lhsT=ga16[:, 256 * b:256 * (b + 1):2],
                                     rhs=sgn_sb[:, 0:1], start=True, stop=False)
                    nc.tensor.matmul(sa_ps[:, g:g + 1],
                                     lhsT=ga16[:, 256 * b + 1:256 * (b + 1):2],
                                     rhs=sgn_sb[:, 1:2], start=False, stop=True)

            def g_window(w):
                wa = w * WIN
                y0t = psw.tile([128, WIN], FP32, tag="y0")
                y1t = psw.tile([128, WIN], FP32, tag="y1")
                y0 = y0t[:]
                y1 = y1t[:]
                for (lo, hi, c1, c2) in segs_by_win[w]:
                    nc.tensor.matmul(y0[:, lo:hi], lhsT=san[:, c1, 0:128],
                                     rhs=ohA_sb[:, wa + lo:wa + hi], start=True, stop=False)
                    nc.tensor.matmul(y0[:, lo:hi], lhsT=scn[:, c2, 0:128],
                                     rhs=ohC_sb[:, wa + lo:wa + hi], start=False, stop=True)
                    nc.tensor.matmul(y1[:, lo:hi], lhsT=san[:, c1, 128:256],
                                     rhs=ohA_sb[:, wa + lo:wa + hi], start=True, stop=False)
                    nc.tensor.matmul(y1[:, lo:hi], lhsT=scn[:, c2, 128:256],
                                     rhs=ohC_sb[:, wa + lo:wa + hi], start=False, stop=True)
                if wz[w]:
                    # fused path: z1 = relu(y1) on Act; zsum = relu(y0)+z1 on DVE
                    z1 = hpool.tile([128, WIN], FP16, tag="z1")
                    nc.scalar.activation(z1[:], y1, AF.Relu)
                    zs = hpool.tile([128, WIN], FP16, tag="zs")
                    nc.vector.scalar_tensor_tensor(zs[:], y0, 0.0, z1[:],
                                                   op0=ALU.max, op1=ALU.add)
                    for b in range(WIN // 128):
                        g = wa // 128 + b
                        sl = slice(b * 128, (b + 1) * 128)
                        nc.tensor.matmul(sg_ps[:, g:g + 1], lhsT=zs[:, sl],
                                         rhs=sgn_sb[:, 0:1], start=True, stop=True)
                else:
                    hg = hpool.tile([128, 2, WIN], FP16, tag="hg")
                    nc.scalar.activation(hg[:, 0, :], y0, AF.Relu)
                    nc.scalar.activation(hg[:, 1, :], y1, AF.Relu)
                    for b in range(WIN // 128):
                        g = wa // 128 + b
                        sl = slice(b * 128, (b + 1) * 128)
                        nc.tensor.matmul(sg_ps[:, g:g + 1], lhsT=hg[:, 0, sl],
                                         rhs=sgn_sb[:, 0:1], start=True, stop=False)
                        nc.tensor.matmul(sg_ps[:, g:g + 1], lhsT=hg[:, 1, sl],
                                         rhs=sgn_sb[:, 1:2], start=False, stop=True)

            # interleave: a few windows first, alpha tiles spread between
            order = []
            wi = 0
            for ai in range(nat):
                take = max(1, nwin // nat if ai < nat - 1 else nwin - wi)
                for _ in range(take):
                    if wi < nwin:
                        order.append(("w", wi)); wi += 1
                order.append(("a", ai))
            while wi < nwin:
                order.append(("w", wi)); wi += 1
            for i in range(nat - 2):
                order.append(("d", i))

            # Z windows spread evenly (rest are P: both relus on Act)
            for i in range(nwin):
                wz[i] = False
            nzw = int(round(ZFRAC * nwin))
            stepz = nwin / max(nzw, 1)
            for i in range(nzw):
                wz[min(int(i * stepz), nwin - 1)] = True

            for kind, i in order:
                if kind == "a":
                    ap_tile(i)
                elif kind == "d":
                    ap_dots(i)
                else:
                    g_window(i)

            psw.release()

            # ---- gamma sigmoid + routing (before the last alpha dots) ----
            scat = cpool.tile([128, W], FP16)
            sg16 = cpool.tile([128, max(Gg, 2)], FP16)
            nc.scalar.activation(sg16[:, 0:Gg], sg_ps[:, 0:Gg], AF.Sigmoid, bias=b2_sb[:, 0:1])
            psr = tc.alloc_tile_pool(name="psr", bufs=3, space="PSUM")
            pst = psr.tile([128, 128], FP16, tag="t0")
            nc.tensor.transpose(pst[0:Gg, :], sg16[:, 0:Gg], id_sb)
            sT = cpool.tile([128, 128], FP16)
            nc.vector.memset(sT[:], 0)
            nc.vector.tensor_copy(sT[0:Gg, :], pst[0:Gg, :])
            panels = cpool.tile([128, mu * 128], FP16)
            nc.gpsimd.local_scatter(panels[:].bitcast(I16), sT[:].bitcast(I16),
                                    rt1_sb, channels=128,
                                    num_elems=mu * 128, num_idxs=128)

            # last alpha dots + alpha sigmoid
            for i in range(nat - 2, nat):
                ap_dots(i)
            nc.scalar.activation(scat[:, 0:Ga], sa_ps[:, 0:Ga], AF.Sigmoid,
                                 bias=b2_sb[:, 0:1])

            for k in range(mu):
                pk = psr.tile([128, 128], FP16, tag="tk")
                nc.tensor.transpose(pk[:], panels[:, k * 128:(k + 1) * 128], id_sb)
                nc.vector.tensor_copy(scat[:, Ga + k * 128:Ga + (k + 1) * 128], pk[:])

            # ---- dense output: scatter, upconvert, store ----
            for c in range(4):
                d16 = wpool.tile([128, N], FP16, tag=f"d16_{c % 2}")
                nc.gpsimd.local_scatter(d16[:].bitcast(I16), scat[:].bitcast(I16),
                                        ls_sb[:, c * W:(c + 1) * W],
                                        channels=128, num_elems=N, num_idxs=W)
                d32 = wpool.tile([128, N], FP32, tag=f"d32_{c % 2}")
                nc.vector.tensor_copy(d32[:], d16[:])
                nc.sync.dma_start(out=out_ext[c * 128:(c + 1) * 128, :], in_=d32[:])
            psr.release()
            psspool.release()

    nc.compile()
    return nc


def _prep_host(features, W1, b1, W2, b2, edge_index):
    f = np.asarray(features, dtype=np.float32)
    W1 = np.asarray(W1, dtype=np.float32)
    b1 = np.asarray(b1, dtype=np.float32)
    W2 = np.asarray(W2, dtype=np.float32)
    b2 = np.asarray(b2, dtype=np.float32)
    ei = np.asarray(edge_index).astype(np.int64)
    src, dst = ei[0], ei[1]

    # ---- sign-pairing permutation + |w2| prescale ----
    w2v = W2[:, 0]
    pos = np.nonzero(w2v >= 0)[0]
    neg = np.nonzero(w2v < 0)[0]
    if len(pos) % 2 == 1:
        m = int(np.argmin(np.abs(w2v)))
        if (w2v[m] >= 0):
            pos = pos[pos != m]
            neg = np.append(neg, m)
        else:
            neg = neg[neg != m]
            pos = np.append(pos, m)
    a_n = len(pos) // 2
    b_n = len(neg) // 2
    assert a_n + b_n == 128, (a_n, b_n)
    perm0 = np.concatenate([pos[:a_n], neg[:b_n]])
    perm1 = np.concatenate([pos[a_n:], neg[b_n:]])
    perm = np.concatenate([perm0, perm1]).astype(np.int64)
    sgn = np.concatenate([np.ones(a_n), -np.ones(b_n)]).astype(np.float16)
    absw = np.abs(w2v[perm])
    W1p = (W1[:, perm] * absw[None, :]).astype(np.float32)
    b1p = (b1[perm] * absw).astype(np.float32)

    f16 = f.astype(np.float16)
    w1r = np.ascontiguousarray(
        W1p.reshape(4, 128, D).transpose(1, 0, 2)).astype(np.float16)
    b1c = np.ascontiguousarray(b1p.reshape(2, 128).T).astype(np.float32)
    b1r = np.ascontiguousarray(
        np.broadcast_to(b1p[None, :], (128, D))).astype(np.float16)
    b2t = np.full((128, 1), b2.reshape(-1)[0], dtype=np.float32)
    sgn2 = np.stack([sgn, sgn], axis=1)  # [128, 2]
    ident = np.eye(128, dtype=np.float16)

    # dedup (keep last)
    flat = src * N + dst
    keep = np.zeros(E, dtype=bool)
    _, first_of_rev = np.unique(flat[::-1], return_index=True)
    keep[E - 1 - first_of_rev] = True
    ks, kd = src[keep], dst[keep]

    part = (ks % 128).astype(np.int64)
    order = np.argsort(part, kind="stable")
    part_s, ks_s, kd_s = part[order], ks[order], kd[order]
    counts = np.bincount(part_s, minlength=128)
    starts = np.zeros(129, np.int64)
    np.cumsum(counts, out=starts[1:])

    Ga = min(GA_TARGET, int(counts.min()) // 16 * 16)
    Ga = max(Ga, 16)
    Sa = 128 * Ga

    # alpha: first Ga edges of each bucket; gamma: the rest
    a_src = np.zeros((128, Ga), np.int64)
    a_dst = np.zeros((128, Ga), np.int64)
    g_list = []
    for p in range(128):
        lo, hi = starts[p], starts[p + 1]
        a_src[p] = ks_s[lo:lo + Ga]
        a_dst[p] = kd_s[lo:lo + Ga]
        g_list.append((ks_s[lo + Ga:hi], kd_s[lo + Ga:hi]))
    gs = np.concatenate([x[0] for x in g_list])
    gd = np.concatenate([x[1] for x in g_list])

    # gamma sorted by (src//128, dst//128), padded with (511,511) dummies.
    gkey = (gs >> 7) * 4 + (gd >> 7)
    gorder = np.argsort(gkey, kind="stable")
    gs, gd = gs[gorder], gd[gorder]
    gkey = gkey[gorder]
    qv = gs % 128
    frac = np.zeros(len(gs), np.float64)
    for k in range(16):
        m = np.nonzero(gkey == k)[0]
        qm = qv[m]
        cnt = {}
        tot = {}
        for qq in qm:
            tot[qq] = tot.get(qq, 0) + 1
        for i, qq in enumerate(qm):
            r = cnt.get(qq, 0)
            cnt[qq] = r + 1
            frac[m[i]] = (r + 0.5) / tot[qq]
    gorder2 = np.lexsort((qv, frac, gkey))
    gs, gd = gs[gorder2], gd[gorder2]
    nreal = len(gs)

    # repair: cap per-(block, target-partition) collisions at 3
    gk2 = ((gs >> 7) * 4 + (gd >> 7)).astype(np.int64)
    for _ in range(6):
        q2 = (gs % 128).astype(np.int64)
        blk2 = np.arange(nreal) // 128
        cnt = np.zeros((nreal // 128 + 1, 128), np.int64)
        np.add.at(cnt, (blk2, q2), 1)
        bad = np.nonzero(cnt[blk2, q2] > 3)[0]
        if len(bad) == 0:
            break
        changed = False
        for t in bad:
            if cnt[blk2[t], q2[t]] <= 3:
                continue
            grp = np.nonzero(gk2 == gk2[t])[0]
            for j in grp:
                bi, bj, qi, qj = blk2[t], blk2[j], q2[t], q2[j]
                if bi == bj:
                    continue
                if cnt[bj, qi] < 3 and cnt[bi, qj] < 3:
                    gs[t], gs[j] = gs[j].copy(), gs[t].copy()
                    gd[t], gd[j] = gd[j].copy(), gd[t].copy()
                    cnt[bi, qi] -= 1; cnt[bj, qi] += 1
                    cnt[bj, qj] -= 1; cnt[bi, qj] += 1
                    q2[t], q2[j] = q2[j], q2[t]
                    changed = True
                    break
        if not changed:
            break
    Sg = max(WIN, (nreal + WIN - 1) // WIN * WIN)
    pad = Sg - nreal
    gs = np.concatenate([gs, np.full(pad, N - 1, np.int64)])
    gd = np.concatenate([gd, np.full(pad, N - 1, np.int64)])
    Gg = Sg // 128
    assert Gg <= 128, f"gamma region too large: {Sg}"

    # segments per window
    gkey = (gs >> 7) * 4 + (gd >> 7)
    segs = []
    t = 0
    while t < Sg:
        w = t // WIN
        wend = (w + 1) * WIN
        k = gkey[t]
        e = t
        while e < wend and gkey[e] == k:
            e += 1
        segs.append((w, t - w * WIN, e - w * WIN, int(k) // 4, int(k) % 4))
        t = e

    # one-hots (fp8: 0/1 exact)
    ohA = np.zeros((128, Sg), ml_dtypes.float8_e4m3)
    ohC = np.zeros((128, Sg), ml_dtypes.float8_e4m3)
    tt = np.arange(Sg)
    ohA[gs & 127, tt] = 1.0
    ohC[gd & 127, tt] = 1.0

    # routing tables
    q = (gs % 128).astype(np.int64)
    blk = tt // 128
    r = tt % 128
    rt1 = np.full((128, 128), -1, np.int16)
    kcount = np.zeros((128, 128), np.int64)
    for t_i in range(nreal):
        c, rr, qq = blk[t_i], r[t_i], q[t_i]
        k = kcount[c, qq]
        kcount[c, qq] += 1
        rt1[c, rr] = k * 128 + qq
    mu = int(kcount.max())
    assert mu <= 15, mu
    W_ = Ga + mu * 128

    # final scatter tables [128, 4, W]
    lsfin = np.full((128, 4, W_), -1, np.int16)
    cca = (a_src >> 7)
    for p in range(128):
        for g in range(Ga):
            lsfin[p, cca[p, g], g] = a_dst[p, g]
    for t_i in range(nreal):
        qq = q[t_i]
        c = blk[t_i]
        k = (rt1[c, r[t_i]] - qq) // 128
        lsfin[qq, int(gs[t_i]) >> 7, Ga + k * 128 + c] = gd[t_i]

    def wrap(a):
        a = np.ascontiguousarray(a).astype(np.int16)
        a16 = a.reshape(-1, 16).T
        return np.ascontiguousarray(np.tile(a16, (8, 1)))

    srcw = wrap(a_src.T.reshape(-1))
    dstw = wrap(a_dst.T.reshape(-1))

    # ---- blobs ----
    def i16(x):
        return np.ascontiguousarray(x).view(np.int16).reshape(128, -1)

    blob1a = np.concatenate([
        i16(ident), i16(w1r.reshape(128, -1)),
        i16(b1c), i16(b2t), i16(sgn2.astype(np.float16)),
    ], axis=1)
    assert blob1a.shape[1] == 1160, blob1a.shape
    blob1b = np.concatenate([i16(b1r), srcw, dstw], axis=1)
    assert blob1b.shape[1] == 256 + 2 * (Sa // 16), blob1b.shape
    blob2 = np.concatenate([rt1, lsfin.reshape(128, -1)], axis=1)

    cfg = (Ga, Sg, mu, W_, tuple(segs))
    shared = {"blob1a": blob1a, "blob1b": blob1b, "blob2": blob2,
              "ohA": ohA, "ohC": ohC}
    in_maps = []
    for b_i in range(B):
        # ft[p, k, n] = f[b, n, k*128+p]
        ftb = np.ascontiguousarray(
            f16[b_i].T.reshape(2, 128, N).transpose(1, 0, 2))
        in_maps.append(dict(shared, ft=ftb))
    return cfg, in_maps


def kernel(features, W1, b1, W2, b2, edge_index):
    cfg, in_maps = _prep_host(features, W1, b1, W2, b2, edge_index)
    if cfg not in _cache:
        _cache[cfg] = build_program(cfg)
    nc = _cache[cfg]
    res = run_bass_kernel_spmd(nc, in_maps, list(range(NCORES)))
    out = np.stack([res.results[c]["out"] for c in range(NCORES)], axis=0)
    return out.astype(np.float32)


# revision 39
# speedup vs baseline: 1.0102x; 1.0102x over previous
"""Bass/Trainium2 kernel for nn_CausalWeighting — fp16, two gather paths, v2.

Per batch b (one core per batch):
    s = sigmoid(relu(A[src] + C[dst]) @ w2 + b2)
with tables prescaled by |w2| and hidden dims permuted so paired dims
(d, d+128) share sign(w2); the w2 dot then reduces with a per-partition
sign vector, enabling a fused relu+add (scalar_tensor_tensor) and a single
matmul per 128-slot group on gamma "Z" windows.

Paths:
  * alpha: [128, Ga] grid bucketed by src%128, packed-int32 ap_gather from
    feature-major tables, DVE add+relu, sign-dot matmul pairs -> sa_ps[q, g].
  * gamma: one-hot (fp8) matmuls accumulate A'+C' in PSUM windows; relu via
    Act/DVE; scores land on arbitrary partitions and are routed via
    transpose -> local_scatter panels -> panel transposes into scat.
  * 4 local_scatters build the dense [512,512] fp16 + upconvert + DMA out.

v2 changes vs baseline: single act-table load at t~0 (dummy sigmoid),
merged param DMA blobs, fp8 one-hots in window-chunks, TS=2048 gathers,
Ga rebalanced 72->48, sign-pair/|w2|-prescale trick, Z-windows with fused
STT relu+add and single w2 matmuls.

Sharding: data-parallel over batch, one batch per core (8 cores).
"""

import sys

import numpy as np
import ml_dtypes

if "/opt/trn_rl_repo" not in sys.path:
    sys.path.insert(0, "/opt/trn_rl_repo")

import concourse.bacc as bacc
import concourse.mybir as mybir
import concourse.tile as tile
from concourse.bass_utils import run_bass_kernel_spmd

B, N, D, E = 8, 512, 256, 16384
NCORES = 8
TS = 2048            # alpha slots per tile
WIN = 512            # gamma slots per window
GA_TARGET = 56       # alpha columns (x128 slots)
ZFRAC = 0.5          # fraction of gamma windows using the fused Z path
OH_CHUNKS = 4        # one-hot DMA chunks
FP32 = mybir.dt.float32
FP16 = mybir.dt.float16
FP8 = mybir.dt.float8e4
I32 = mybir.dt.int32
I16 = mybir.dt.int16
AF = mybir.ActivationFunctionType
ALU = mybir.AluOpType

_cache = {}


def build_program(cfg):
    """cfg: (Ga, Sg, mu, W, segs) with segs = tuple of
    (window, col_lo, col_hi, c1, c2) covering [0, Sg)."""
    Ga, Sg, mu, W, segs = cfg
    Sa = 128 * Ga
    assert Sg % WIN == 0
    nwin = Sg // WIN
    Gg = Sg // 128
    assert Gg <= 128
    na = Sa // TS
    nc = bacc.Bacc(None)

    # blob1a layout (int16 columns): critical-path params
    SW = Sa // 16
    o_id = 0          # ident fp16 [128, 128]
    o_w1 = 128        # w1 fp16 [128, 4*256]
    o_b1c = 1152      # b1c fp32 [128, 2] -> 4 cols
    o_b2 = 1156       # b2 fp32 [128, 1] -> 2 cols
    o_sgn = 1158      # sgn fp16 [128, 2] (two equal cols)
    NB1A = 1160
    # blob1b: b1r + gather indices
    o_b1r = 0         # b1r fp16 [128, 256]
    o_srcw = 256      # i16 [128, SW]
    o_dstw = o_srcw + SW
    NB1B = o_dstw + SW
    # blob2 layout
    o_rt1 = 0
    o_ls = 128
    NB2 = 128 + 4 * W

    ft_ext = nc.declare_dram_parameter("ft", [128, 2, N], FP16, isOutput=False)
    b1a_ext = nc.declare_dram_parameter("blob1a", [128, NB1A], I16, isOutput=False)
    b1b_ext = nc.declare_dram_parameter("blob1b", [128, NB1B], I16, isOutput=False)
    b2_ext = nc.declare_dram_parameter("blob2", [128, NB2], I16, isOutput=False)
    ohA_ext = nc.declare_dram_parameter("ohA", [128, Sg], FP8, isOutput=False)
    ohC_ext = nc.declare_dram_parameter("ohC", [128, Sg], FP8, isOutput=False)
    out_ext = nc.declare_dram_parameter("out", [N, N], FP32, isOutput=True)

    # window strategies: True = Z (fused STT+single matmuls); P windows
    # (both relus on Act) are assigned next to alpha DVE pieces, below.
    wz = [True] * nwin

    segs_by_win = [[] for _ in range(nwin)]
    for (w, lo, hi, c1, c2) in segs:
        segs_by_win[w].append((lo, hi, c1, c2))

    # alpha tile sizes: trailing tiles smaller for a short tail
    assert Sa % 1024 == 0
    nbig = max(0, (Sa - 2048) // 2048)
    tsz = [2048] * nbig + [1024] * ((Sa - 2048 * nbig) // 1024)
    t0s = np.cumsum([0] + tsz).tolist()
    nat = len(tsz)

    from contextlib import ExitStack
    with tile.TileContext(nc) as tc:
        with (
            tc.tile_pool(name="const", bufs=1) as cpool,
            tc.tile_pool(name="work", bufs=2) as wpool,
            tc.tile_pool(name="gwork", bufs=nat) as gpool,
            tc.tile_pool(name="hwork", bufs=4) as hpool,
        ):
            # ---- act-table preload: dummy sigmoid first ----
            dummy = cpool.tile([128, 8], FP16)
            nc.vector.memset(dummy[:], 0)
            nc.scalar.activation(dummy[:], dummy[:], AF.Sigmoid)

            # ---- PE warmup: dep-free dummy matmuls ramp the p-state so the
            # real table matmuls start at full clock (3us continuous rule)
            wmt = cpool.tile([128, N], FP16)
            nc.vector.memset(wmt[:], 0)

            # ---- loads ----
            ft_sb = cpool.tile([128, 2, N], FP16)
            nc.sync.dma_start(out=ft_sb[:], in_=ft_ext[:])
            bl1a = cpool.tile([128, NB1A], I16)
            nc.sync.dma_start(out=bl1a[:], in_=b1a_ext[:])
            bl1b = cpool.tile([128, NB1B], I16)
            nc.sync.dma_start(out=bl1b[:], in_=b1b_ext[:])
            ohA_sb = cpool.tile([128, Sg], FP8)
            ohC_sb = cpool.tile([128, Sg], FP8)
            ch = (nwin + OH_CHUNKS - 1) // OH_CHUNKS
            for c0 in range(0, nwin, ch):
                sl = slice(c0 * WIN, min((c0 + ch) * WIN, Sg))
                nc.sync.dma_start(out=ohA_sb[:, sl], in_=ohA_ext[:, sl])
                nc.sync.dma_start(out=ohC_sb[:, sl], in_=ohC_ext[:, sl])
            bl2 = cpool.tile([128, NB2], I16)
            nc.sync.dma_start(out=bl2[:], in_=b2_ext[:])

            id_sb = bl1a[:, o_id:o_id + 128].bitcast(FP16)
            b1c_sb = bl1a[:, o_b1c:o_b1c + 4].bitcast(FP32)
            b2_sb = bl1a[:, o_b2:o_b2 + 2].bitcast(FP32)
            sgn_sb = bl1a[:, o_sgn:o_sgn + 2].bitcast(FP16)
            b1r_sb = bl1b[:, o_b1r:o_b1r + 256].bitcast(FP16)
            srcw_sb = bl1b[:, o_srcw:o_srcw + SW]
            dstw_sb = bl1b[:, o_dstw:o_dstw + SW]
            rt1_sb = bl2[:, o_rt1:o_rt1 + 128]
            ls_sb = bl2[:, o_ls:o_ls + 4 * W]

            def w1_blk(k, lo, hi):
                return bl1a[:, o_w1 + k * 256 + lo:o_w1 + k * 256 + hi].bitcast(FP16)

            setup_ps = ExitStack()
            pspool = setup_ps.enter_context(
                tc.tile_pool(name="ps", bufs=2, space="PSUM"))

            for _ in range(7):
                wps = pspool.tile([128, N], FP32, tag="tb")
                nc.tensor.matmul(wps[:], lhsT=wmt[:, 0:128], rhs=wmt[:],
                                 start=True, stop=True)

            # ---- feature-major packed tables (alpha); at16 first: it gates
            # the gathers ----
            at16 = cpool.tile([128, 2 * N], FP16)
            ct16 = cpool.tile([128, 2 * N], FP16)
            for m in range(2):
                psa = pspool.tile([128, N], FP32, tag="tb")
                for k in range(2):
                    nc.tensor.matmul(psa[:], lhsT=w1_blk(k, m * 128, (m + 1) * 128),
                                     rhs=ft_sb[:, k, :], start=(k == 0), stop=(k == 1))
                nc.scalar.activation(at16[:, m::2], psa[:], AF.Identity,
                                     bias=b1c_sb[:, m:m + 1])
            for m in range(2):
                psc = pspool.tile([128, N], FP32, tag="tb")
                for k in range(2):
                    nc.tensor.matmul(psc[:], lhsT=w1_blk(2 + k, m * 128, (m + 1) * 128),
                                     rhs=ft_sb[:, k, :], start=(k == 0), stop=(k == 1))
                nc.scalar.activation(ct16[:, m::2], psc[:], AF.Identity)

            # ---- node-major tables in SBUF (gamma lhsT) ----
            san = cpool.tile([128, 4, D], FP16)
            scn = cpool.tile([128, 4, D], FP16)
            for c in range(4):
                psn = pspool.tile([128, D], FP32, tag="nm")
                for k in range(2):
                    nc.tensor.matmul(psn[:], lhsT=ft_sb[:, k, c * 128:(c + 1) * 128],
                                     rhs=w1_blk(k, 0, 256), start=(k == 0), stop=(k == 1))
                nc.vector.tensor_add(san[:, c, :], psn[:], b1r_sb)
                psn2 = pspool.tile([128, D], FP32, tag="nm")
                for k in range(2):
                    nc.tensor.matmul(psn2[:], lhsT=ft_sb[:, k, c * 128:(c + 1) * 128],
                                     rhs=w1_blk(2 + k, 0, 256), start=(k == 0), stop=(k == 1))
                nc.vector.tensor_copy(scn[:, c, :], psn2[:])
            atP = at16[:].bitcast(I32)
            ctP = ct16[:].bitcast(I32)

            setup_ps.close()
            psspool = tc.alloc_tile_pool(name="pss", bufs=1, space="PSUM")
            sa_tile = psspool.tile([128, max(Ga, 2)], FP32, tag="sa")
            sg_tile = psspool.tile([128, max(Gg, 2)], FP32, tag="sg")
            sa_ps = sa_tile[:]
            sg_ps = sg_tile[:]
            psw = tc.alloc_tile_pool(name="psw", bufs=3, space="PSUM")

            alpha_relu = []

            alpha_g = []

            def ap_tile(i):
                t0, ts = t0s[i], tsz[i]
                csl = slice(t0 // 16, (t0 + ts) // 16)
                ga = gpool.tile([128, TS], I32, tag="ga")
                gc = gpool.tile([128, TS], I32, tag="gc")
                nc.gpsimd.ap_gather(ga[:, 0:ts], atP, srcw_sb[:, csl],
                                    channels=128, num_elems=N, d=1, num_idxs=ts)
                nc.gpsimd.ap_gather(gc[:, 0:ts], ctP, dstw_sb[:, csl],
                                    channels=128, num_elems=N, d=1, num_idxs=ts)
                ga16 = ga[:, 0:ts].bitcast(FP16)
                gc16 = gc[:, 0:ts].bitcast(FP16)
                nc.vector.tensor_add(ga16, ga16, gc16)
                nc.vector.tensor_scalar_max(ga16, ga16, 0.0)
                alpha_relu.append(ga16)

            def ap_dots(i):
                # deferred: emitted after all windows so the in-order PE
                # stream never stalls on DVE behind these
                t0, ts = t0s[i], tsz[i]
                ga16 = alpha_relu[i]
                for b in range(ts // 128):
                    g = t0 // 128 + b
                    nc.tensor.matmul(sa_ps[:, g:g + 1],
                                     lhsT=ga16[:, 256 * b:256 * (b + 1):2],
                                     rhs=sgn_sb[:, 0:1], start=True, stop=False)
                    nc.tensor.matmul(sa_ps[:, g:g + 1],
                                     lhsT=ga16[:, 256 * b + 1:256 * (b + 1):2],
                                     rhs=sgn_sb[:, 1:2], start=False, stop=True)

            def g_window(w):
                wa = w * WIN
                y0t = psw.tile([128, WIN], FP32, tag="y0")
                y1t = psw.tile([128, WIN], FP32, tag="y1")
                y0 = y0t[:]
                y1 = y1t[:]
                for (lo, hi, c1, c2) in segs_by_win[w]:
                    nc.tensor.matmul(y0[:, lo:hi], lhsT=san[:, c1, 0:128],
                                     rhs=ohA_sb[:, wa + lo:wa + hi], start=True, stop=False)
                    nc.tensor.matmul(y0[:, lo:hi], lhsT=scn[:, c2, 0:128],
                                     rhs=ohC_sb[:, wa + lo:wa + hi], start=False, stop=True)
                    nc.tensor.matmul(y1[:, lo:hi], lhsT=san[:, c1, 128:256],
                                     rhs=ohA_sb[:, wa + lo:wa + hi], start=True, stop=False)
                    nc.tensor.matmul(y1[:, lo:hi], lhsT=scn[:, c2, 128:256],
                                     rhs=ohC_sb[:, wa + lo:wa + hi], start=False, stop=True)
                if wz[w]:
                    # fused path: z1 = relu(y1) on Act; zsum = relu(y0)+z1 on DVE
                    z1 = hpool.tile([128, WIN], FP16, tag="z1")
                    nc.scalar.activation(z1[:], y1, AF.Relu)
                    zs = hpool.tile([128, WIN], FP16, tag="zs")
                    nc.vector.scalar_tensor_tensor(zs[:], y0, 0.0, z1[:],
                                                   op0=ALU.max, op1=ALU.add)
                    for b in range(WIN // 128):
                        g = wa // 128 + b
                        sl = slice(b * 128, (b + 1) * 128)
                        nc.tensor.matmul(sg_ps[:, g:g + 1], lhsT=zs[:, sl],
                                         rhs=sgn_sb[:, 0:1], start=True, stop=True)
                else:
                    hg = hpool.tile([128, 2, WIN], FP16, tag="hg")
                    nc.scalar.activation(hg[:, 0, :], y0, AF.Relu)
                    nc.scalar.activation(hg[:, 1, :], y1, AF.Relu)
                    for b in range(WIN // 128):
                        g = wa // 128 + b
                        sl = slice(b * 128, (b + 1) * 128)
                        nc.tensor.matmul(sg_ps[:, g:g + 1], lhsT=hg[:, 0, sl],
                                         rhs=sgn_sb[:, 0:1], start=True, stop=False)
                        nc.tensor.matmul(sg_ps[:, g:g + 1], lhsT=hg[:, 1, sl],
                                         rhs=sgn_sb[:, 1:2], start=False, stop=True)

            # interleave: a few windows first, alpha tiles spread between
            order = []
            wi = 0
            for ai in range(nat):
                take = max(1, nwin // nat if ai < nat - 1 else nwin - wi)
                for _ in range(take):
                    if wi < nwin:
                        order.append(("w", wi)); wi += 1
                order.append(("a", ai))
            while wi < nwin:
                order.append(("w", wi)); wi += 1
            for i in range(nat - 2):
                order.append(("d", i))

            # Z windows spread evenly (rest are P: both relus on Act)
            for i in range(nwin):
                wz[i] = False
            nzw = int(round(ZFRAC * nwin))
            stepz = nwin / max(nzw, 1)
            for i in range(nzw):
                wz[min(int(i * stepz), nwin - 1)] = True

            for kind, i in order:
                if kind == "a":
                    ap_tile(i)
                elif kind == "d":
                    ap_dots(i)
                else:
                    g_window(i)

            psw.release()

            # ---- gamma sigmoid + routing (before the last alpha dots) ----
            scat = cpool.tile([128, W], FP16)
            sg16 = cpool.tile([128, max(Gg, 2)], FP16)
            nc.scalar.activation(sg16[:, 0:Gg], sg_ps[:, 0:Gg], AF.Sigmoid, bias=b2_sb[:, 0:1])
            psr = tc.alloc_tile_pool(name="psr", bufs=3, space="PSUM")
            pst = psr.tile([128, 128], FP16, tag="t0")
            nc.tensor.transpose(pst[0:Gg, :], sg16[:, 0:Gg], id_sb)
            sT = cpool.tile([128, 128], FP16)
            nc.vector.memset(sT[:], 0)
            nc.vector.tensor_copy(sT[0:Gg, :], pst[0:Gg, :])
            panels = cpool.tile([128, mu * 128], FP16)
            nc.gpsimd.local_scatter(panels[:].bitcast(I16), sT[:].bitcast(I16),
                                    rt1_sb, channels=128,
                                    num_elems=mu * 128, num_idxs=128)

            # last alpha dots + alpha sigmoid
            for i in range(nat - 2, nat):
                ap_dots(i)
            nc.scalar.activation(scat[:, 0:Ga], sa_ps[:, 0:Ga], AF.Sigmoid,
                                 bias=b2_sb[:, 0:1])

            for k in range(mu):
                pk = psr.tile([128, 128], FP16, tag="tk")
                nc.tensor.transpose(pk[:], panels[:, k * 128:(k + 1) * 128], id_sb)
                nc.vector.tensor_copy(scat[:, Ga + k * 128:Ga + (k + 1) * 128], pk[:])

            # ---- dense output: scatter, upconvert, store ----
            for c in range(4):
                d16 = wpool.tile([128, N], FP16, tag=f"d16_{c % 2}")
                nc.gpsimd.local_scatter(d16[:].bitcast(I16), scat[:].bitcast(I16),
                                        ls_sb[:, c * W:(c + 1) * W],
                                        channels=128, num_elems=N, num_idxs=W)
                d32 = wpool.tile([128, N], FP32, tag=f"d32_{c % 2}")
                nc.vector.tensor_copy(d32[:], d16[:])
                nc.sync.dma_start(out=out_ext[c * 128:(c + 1) * 128, :], in_=d32[:])
            psr.release()
            psspool.release()

    nc.compile()
    return nc


def _prep_host(features, W1, b1, W2, b2, edge_index):
    f = np.asarray(features, dtype=np.float32)
    W1 = np.asarray(W1, dtype=np.float32)
    b1 = np.asarray(b1, dtype=np.float32)
    W2 = np.asarray(W2, dtype=np.float32)
    b2 = np.asarray(b2, dtype=np.float32)
    ei = np.asarray(edge_index).astype(np.int64)
    src, dst = ei[0], ei[1]

    # ---- sign-pairing permutation + |w2| prescale ----
    w2v = W2[:, 0]
    pos = np.nonzero(w2v >= 0)[0]
    neg = np.nonzero(w2v < 0)[0]
    if len(pos) % 2 == 1:
        m = int(np.argmin(np.abs(w2v)))
        if (w2v[m] >= 0):
            pos = pos[pos != m]
            neg = np.append(neg, m)
        else:
            neg = neg[neg != m]
            pos = np.append(pos, m)
    a_n = len(pos) // 2
    b_n = len(neg) // 2
    assert a_n + b_n == 128, (a_n, b_n)
    perm0 = np.concatenate([pos[:a_n], neg[:b_n]])
    perm1 = np.concatenate([pos[a_n:], neg[b_n:]])
    perm = np.concatenate([perm0, perm1]).astype(np.int64)
    sgn = np.concatenate([np.ones(a_n), -np.ones(b_n)]).astype(np.float16)
    absw = np.abs(w2v[perm])
    W1p = (W1[:, perm] * absw[None, :]).astype(np.float32)
    b1p = (b1[perm] * absw).astype(np.float32)

    f16 = f.astype(np.float16)
    w1r = np.ascontiguousarray(
        W1p.reshape(4, 128, D).transpose(1, 0, 2)).astype(np.float16)
    b1c = np.ascontiguousarray(b1p.reshape(2, 128).T).astype(np.float32)
    b1r = np.ascontiguousarray(
        np.broadcast_to(b1p[None, :], (128, D))).astype(np.float16)
    b2t = np.full((128, 1), b2.reshape(-1)[0], dtype=np.float32)
    sgn2 = np.stack([sgn, sgn], axis=1)  # [128, 2]
    ident = np.eye(128, dtype=np.float16)

    # dedup (keep last)
    flat = src * N + dst
    keep = np.zeros(E, dtype=bool)
    _, first_of_rev = np.unique(flat[::-1], return_index=True)
    keep[E - 1 - first_of_rev] = True
    ks, kd = src[keep], dst[keep]

    part = (ks % 128).astype(np.int64)
    order = np.argsort(part, kind="stable")
    part_s, ks_s, kd_s = part[order], ks[order], kd[order]
    counts = np.bincount(part_s, minlength=128)
    starts = np.zeros(129, np.int64)
    np.cumsum(counts, out=starts[1:])

    Ga = min(GA_TARGET, int(counts.min()) // 16 * 16)
    Ga = max(Ga, 16)
    Sa = 128 * Ga

    # alpha: first Ga edges of each bucket; gamma: the rest
    a_src = np.zeros((128, Ga), np.int64)
    a_dst = np.zeros((128, Ga), np.int64)
    g_list = []
    for p in range(128):
        lo, hi = starts[p], starts[p + 1]
        a_src[p] = ks_s[lo:lo + Ga]
        a_dst[p] = kd_s[lo:lo + Ga]
        g_list.append((ks_s[lo + Ga:hi], kd_s[lo + Ga:hi]))
    gs = np.concatenate([x[0] for x in g_list])
    gd = np.concatenate([x[1] for x in g_list])

    # gamma sorted by (src//128, dst//128), padded with (511,511) dummies.
    gkey = (gs >> 7) * 4 + (gd >> 7)
    gorder = np.argsort(gkey, kind="stable")
    gs, gd = gs[gorder], gd[gorder]
    gkey = gkey[gorder]
    qv = gs % 128
    frac = np.zeros(len(gs), np.float64)
    for k in range(16):
        m = np.nonzero(gkey == k)[0]
        qm = qv[m]
        cnt = {}
        tot = {}
        for qq in qm:
            tot[qq] = tot.get(qq, 0) + 1
        for i, qq in enumerate(qm):
            r = cnt.get(qq, 0)
            cnt[qq] = r + 1
            frac[m[i]] = (r + 0.5) / tot[qq]
    gorder2 = np.lexsort((qv, frac, gkey))
    gs, gd = gs[gorder2], gd[gorder2]
    nreal = len(gs)

    # repair: cap per-(block, target-partition) collisions at 3
    gk2 = ((gs >> 7) * 4 + (gd >> 7)).astype(np.int64)
    for _ in range(6):
        q2 = (gs % 128).astype(np.int64)
        blk2 = np.arange(nreal) // 128
        cnt = np.zeros((nreal // 128 + 1, 128), np.int64)
        np.add.at(cnt, (blk2, q2), 1)
        bad = np.nonzero(cnt[blk2, q2] > 3)[0]
        if len(bad) == 0:
            break
        changed = False
        for t in bad:
            if cnt[blk2[t], q2[t]] <= 3:
                continue
            grp = np.nonzero(gk2 == gk2[t])[0]
            for j in grp:
                bi, bj, qi, qj = blk2[t], blk2[j], q2[t], q2[j]
                if bi == bj:
                    continue
                if cnt[bj, qi] < 3 and cnt[bi, qj] < 3:
                    gs[t], gs[j] = gs[j].copy(), gs[t].copy()
                    gd[t], gd[j] = gd[j].copy(), gd[t].copy()
                    cnt[bi, qi] -= 1; cnt[bj, qi] += 1
                    cnt[bj, qj] -= 1; cnt[bi, qj] += 1
                    q2[t], q2[j] = q2[j], q2[t]
                    changed = True
                    break
        if not changed:
            break
    Sg = max(WIN, (nreal + WIN - 1) // WIN * WIN)
    pad = Sg - nreal
    gs = np.concatenate([gs, np.full(pad, N - 1, np.int64)])
    gd = np.concatenate([gd, np.full(pad, N - 1, np.int64)])
    Gg = Sg // 128
    assert Gg <= 128, f"gamma region too large: {Sg}"

    # segments per window
    gkey = (gs >> 7) * 4 + (gd >> 7)
    segs = []
    t = 0
    while t < Sg:
        w = t // WIN
        wend = (w + 1) * WIN
        k = gkey[t]
        e = t
        while e < wend and gkey[e] == k:
            e += 1
        segs.append((w, t - w * WIN, e - w * WIN, int(k) // 4, int(k) % 4))
        t = e

    # one-hots (fp8: 0/1 exact)
    ohA = np.zeros((128, Sg), ml_dtypes.float8_e4m3)
    ohC = np.zeros((128, Sg), ml_dtypes.float8_e4m3)
    tt = np.arange(Sg)
    ohA[gs & 127, tt] = 1.0
    ohC[gd & 127, tt] = 1.0

    # routing tables
    q = (gs % 128).astype(np.int64)
    blk = tt // 128
    r = tt % 128
    rt1 = np.full((128, 128), -1, np.int16)
    kcount = np.zeros((128, 128), np.int64)
    for t_i in range(nreal):
        c, rr, qq = blk[t_i], r[t_i], q[t_i]
        k = kcount[c, qq]
        kcount[c, qq] += 1
        rt1[c, rr] = k * 128 + qq
    mu = int(kcount.max())
    assert mu <= 15, mu
    W_ = Ga + mu * 128

    # final scatter tables [128, 4, W]
    lsfin = np.full((128, 4, W_), -1, np.int16)
    cca = (a_src >> 7)
    for p in range(128):
        for g in range(Ga):
            lsfin[p, cca[p, g], g] = a_dst[p, g]
    for t_i in range(nreal):
        qq = q[t_i]
        c = blk[t_i]
        k = (rt1[c, r[t_i]] - qq) // 128
        lsfin[qq, int(gs[t_i]) >> 7, Ga + k * 128 + c] = gd[t_i]

    def wrap(a):
        a = np.ascontiguousarray(a).astype(np.int16)
        a16 = a.reshape(-1, 16).T
        return np.ascontiguousarray(np.tile(a16, (8, 1)))

    srcw = wrap(a_src.T.reshape(-1))
    dstw = wrap(a_dst.T.reshape(-1))

    # ---- blobs ----
    def i16(x):
        return np.ascontiguousarray(x).view(np.int16).reshape(128, -1)

    blob1a = np.concatenate([
        i16(ident), i16(w1r.reshape(128, -1)),
        i16(b1c), i16(b2t), i16(sgn2.astype(np.float16)),
    ], axis=1)
    assert blob1a.shape[1] == 1160, blob1a.shape
    blob1b = np.concatenate([i16(b1r), srcw, dstw], axis=1)
    assert blob1b.shape[1] == 256 + 2 * (Sa // 16), blob1b.shape
    blob2 = np.concatenate([rt1, lsfin.reshape(128, -1)], axis=1)

    cfg = (Ga, Sg, mu, W_, tuple(segs))
    shared = {"blob1a": blob1a, "blob1b": blob1b, "blob2": blob2,
              "ohA": ohA, "ohC": ohC}
    in_maps = []
    for b_i in range(B):
        # ft[p, k, n] = f[b, n, k*128+p]
        ftb = np.ascontiguousarray(
            f16[b_i].T.reshape(2, 128, N).transpose(1, 0, 2))
        in_maps.append(dict(shared, ft=ftb))
    return cfg, in_maps


def kernel(features, W1, b1, W2, b2, edge_index):
    cfg, in_maps = _prep_host(features, W1, b1, W2, b2, edge_index)
    if cfg not in _cache:
        _cache[cfg] = build_program(cfg)
    nc = _cache[cfg]
    res = run_bass_kernel_spmd(nc, in_maps, list(range(NCORES)))
    out = np.stack([res.results[c]["out"] for c in range(NCORES)], axis=0)
    return out.astype(np.float32)
